# revision 6
# baseline (speedup 1.0000x reference)
"""Trainium2 Bass kernel for the ConvE-style MoE-routing block.

Computes, for each batch row b:
    X = [e1|e2] @ rel_emb.T            # [B, NR] gating logits
    S, idx = top_k(sigmoid(X), 16)
    R1 = relu(rel_emb @ W_fcs.T + b)   # [NR, D]
    out = sum_k S_k * R1[idx_k] / sum_k S_k

Reformulated gather-free: zap the top-16 logits per row with two
(max8 + match_replace) rounds, then M = sigmoid(X) - sigmoid(X_zapped)
is exactly the top-16 sigmoid weights (0 elsewhere), so
    out = (M @ R1) / rowsum(M)
runs on the tensor engine as a dense matmul.

Data-parallel over batch across 8 cores; rel_emb/W_fcs replicated.
"""
import numpy as np

import concourse.bacc as bacc
import concourse.mybir as mybir
from concourse.bass_utils import run_bass_kernel_spmd
from concourse.masks import make_identity
from concourse.tile import TileContext

P = 128
D = 512
TWO_D = 1024
NR = 2048
B = 8192
N_CORES = 8
BC = B // N_CORES      # 1024 batch rows per core
RT = BC // P           # 8 row tiles per core
KC = TWO_D // P        # 8 feature (contraction) chunks
NRC = NR // P          # 16 rel chunks
NBANK = NR // 512      # 4 PSUM banks for one X row-tile
NEG = -60.0            # sigmoid(x - anything <= NEG+max|x|) == 0 to fp32

F32 = mybir.dt.float32
F32R = mybir.dt.float32r
AF = mybir.ActivationFunctionType

_CACHED = None


def _build():
    nc = bacc.Bacc("TRN2", target_bir_lowering=False, debug=True)
    e1 = nc.declare_dram_parameter("e1", [BC, D], F32, isOutput=False)
    e2 = nc.declare_dram_parameter("e2", [BC, D], F32, isOutput=False)
    rel = nc.declare_dram_parameter("rel_emb", [NR, TWO_D], F32, isOutput=False)
    wf = nc.declare_dram_parameter("W_fcs", [D, TWO_D], F32, isOutput=False)
    bf = nc.declare_dram_parameter("b_fcs", [1, D], F32R, isOutput=False)
    out = nc.declare_dram_parameter("out", [BC, D], F32, isOutput=True)

    # Alternate PSUM->SBUF transpose evictions between ACT and DVE to
    # balance engine load.
    evict_ctr = [0]

    def evict(dst, src):
        if evict_ctr[0] % 2 == 0:
            nc.scalar.activation(dst, src, AF.Copy)
        else:
            nc.vector.tensor_copy(dst, src)
        evict_ctr[0] += 1

    with TileContext(nc) as tc:
        with (
            tc.tile_pool(name="consts", bufs=1) as consts,
            tc.tile_pool(name="persist", bufs=1) as persist,
            tc.tile_pool(name="pst", bufs=2, space="PSUM") as pst,
            tc.tile_pool(name="psx", bufs=1, space="PSUM") as psx,
            tc.tile_pool(name="pso", bufs=2, space="PSUM") as pso,
        ):
            ident = consts.tile([P, P], F32)
            make_identity(nc, ident)
            ones1_f32 = consts.tile([1, P], F32)
            nc.vector.memset(ones1_f32, 1.0)
            ones1 = consts.tile([1, P], F32R)
            nc.vector.tensor_copy(ones1, ones1_f32)
            b_sb = consts.tile([1, D], F32R)
            nc.sync.dma_start(out=b_sb, in_=bf[:])

            # R^T: chunk k (features k*128..) lives at cols [k*NR, (k+1)*NR)
            rt_sb = persist.tile([P, KC * NR], F32)
            # W^T: chunk k at cols [k*D, (k+1)*D)
            wt_sb = persist.tile([P, KC * D], F32R)
            # R1: rel-chunk c at cols [c*D, (c+1)*D)
            r1_sb = persist.tile([P, NRC * D], F32R)

            with tc.tile_pool(name="pre", bufs=2) as pre:
                for a in range(D // P):
                    w_tmp = pre.tile([P, TWO_D], F32, tag="w_tmp")
                    nc.sync.dma_start(out=w_tmp, in_=wf[a * P:(a + 1) * P, :])
                    for k in range(KC):
                        pt = pst.tile([P, P], F32)
                        nc.tensor.transpose(pt, w_tmp[:, k * P:(k + 1) * P], ident)
                        evict(wt_sb[:, k * D + a * P: k * D + (a + 1) * P], pt)
                # R^T blocks, evicted twice: fp32 (gating) + f32r (R1 lhsT,
                # since f32r matmul operands must be produced as f32r). The
                # R1 matmuls for rel-chunk c run as soon as its blocks land.
                for c in range(NRC):
                    r_tmp = pre.tile([P, TWO_D], F32, tag="r_tmp")
                    nc.sync.dma_start(out=r_tmp, in_=rel[c * P:(c + 1) * P, :])
                    rstage = pre.tile([P, KC * P], F32R, tag="rstage")
                    for k in range(KC):
                        pt = pst.tile([P, P], F32)
                        nc.tensor.transpose(pt, r_tmp[:, k * P:(k + 1) * P], ident)
                        evict(rt_sb[:, k * NR + c * P: k * NR + (c + 1) * P], pt)
                        evict(rstage[:, k * P:(k + 1) * P], pt)
                    # R1 = relu(R @ W^T + b), float32r (value-grade).
                    pr = pso.tile([P, D], F32, tag="pso")
                    for k in range(KC):
                        nc.tensor.matmul(
                            pr,
                            lhsT=rstage[:, k * P:(k + 1) * P],
                            rhs=wt_sb[:, k * D:(k + 1) * D],
                            start=(k == 0),
                            stop=False,
                        )
                    nc.tensor.matmul(
                        pr, lhsT=ones1, rhs=b_sb, start=False, stop=True,
                    )
                    nc.scalar.activation(r1_sb[:, c * D:(c + 1) * D], pr, AF.Relu)

            with tc.tile_pool(name="work", bufs=2) as work:
                for m in range(RT):
                    st = work.tile([P, TWO_D], F32, tag="st")
                    nc.sync.dma_start(out=st[:, :D], in_=e1[m * P:(m + 1) * P, :])
                    nc.sync.dma_start(out=st[:, D:], in_=e2[m * P:(m + 1) * P, :])
                    # stacked^T: feature-chunk k at cols [k*P, (k+1)*P)
                    stt = work.tile([P, TWO_D], F32, tag="stt")
                    for k in range(KC):
                        pt = pst.tile([P, P], F32)
                        nc.tensor.transpose(pt, st[:, k * P:(k + 1) * P], ident)
                        evict(stt[:, k * P:(k + 1) * P], pt)

                    # Gating X = stacked @ R^T, fp32 (selection-grade).
                    xp = psx.tile([P, NR], F32, tag="xp")
                    for k in range(KC):
                        for nb in range(NBANK):
                            nc.tensor.matmul(
                                xp[:, nb * 512:(nb + 1) * 512],
                                lhsT=stt[:, k * P:(k + 1) * P],
                                rhs=rt_sb[:, k * NR + nb * 512: k * NR + (nb + 1) * 512],
                                start=(k == 0),
                                stop=(k == KC - 1),
                            )
                    xs = work.tile([P, NR], F32, tag="xs")
                    for nb in range(NBANK):
                        nc.scalar.activation(
                            xs[:, nb * 512:(nb + 1) * 512],
                            xp[:, nb * 512:(nb + 1) * 512], AF.Copy,
                        )

                    # Zap top-16 values.
                    m1 = work.tile([P, 8], F32, tag="m1")
                    nc.vector.max(out=m1, in_=xs)
                    xz = work.tile([P, NR], F32, tag="xz")
                    nc.vector.match_replace(
                        out=xz, in_to_replace=m1, in_values=xs, imm_value=NEG)
                    m2 = work.tile([P, 8], F32, tag="m2")
                    nc.vector.max(out=m2, in_=xz)
                    nc.vector.match_replace(
                        out=xz, in_to_replace=m2, in_values=xz, imm_value=NEG)

                    # M = sigmoid(X) - sigmoid(X_zapped); denom via accum.
                    acc_all = work.tile([P, 1], F32, tag="acc_all")
                    nc.scalar.activation(xs, xs, AF.Sigmoid, accum_out=acc_all)
                    acc_exc = work.tile([P, 1], F32, tag="acc_exc")
                    nc.scalar.activation(xz, xz, AF.Sigmoid, accum_out=acc_exc)
                    nc.vector.tensor_sub(xs, xs, xz)
                    den = work.tile([P, 1], F32, tag="den")
                    nc.vector.tensor_sub(den, acc_all, acc_exc)
                    rec = work.tile([P, 1], F32, tag="rec")
                    nc.vector.reciprocal(rec, den)

                    # M^T (rel on partitions): chunk c at cols [c*P, (c+1)*P)
                    mt = work.tile([P, NRC * P], F32R, tag="mt")
                    for c in range(NRC):
                        pt = pst.tile([P, P], F32)
                        nc.tensor.transpose(pt, xs[:, c * P:(c + 1) * P], ident)
                        evict(mt[:, c * P:(c + 1) * P], pt)

                    # Combine: out2 = M @ R1 (float32r), then scale by 1/denom.
                    op = pso.tile([P, D], F32, tag="pso")
                    for c in range(NRC):
                        nc.tensor.matmul(
                            op,
                            lhsT=mt[:, c * P:(c + 1) * P],
                            rhs=r1_sb[:, c * D:(c + 1) * D],
                            start=(c == 0),
                            stop=(c == NRC - 1),
                        )
                    ot = work.tile([P, D], F32, tag="ot")
                    nc.scalar.activation(ot, op, AF.Copy, scale=rec)
                    nc.sync.dma_start(out=out[m * P:(m + 1) * P, :], in_=ot)

    nc.finalize()
    return nc


def _get_nc():
    global _CACHED
    if _CACHED is None:
        _CACHED = _build()
    return _CACHED


def kernel(e1, e2, rel_emb, W_fcs, b_fcs, **_ignored):
    e1 = np.ascontiguousarray(np.asarray(e1, dtype=np.float32))
    e2 = np.ascontiguousarray(np.asarray(e2, dtype=np.float32))
    rel_emb = np.ascontiguousarray(np.asarray(rel_emb, dtype=np.float32))
    W_fcs = np.ascontiguousarray(np.asarray(W_fcs, dtype=np.float32))
    b_fcs = np.ascontiguousarray(
        np.asarray(b_fcs, dtype=np.float32).reshape(1, D))

    nc = _get_nc()
    in_maps = [
        {
            "e1": e1[c * BC:(c + 1) * BC],
            "e2": e2[c * BC:(c + 1) * BC],
            "rel_emb": rel_emb,
            "W_fcs": W_fcs,
            "b_fcs": b_fcs,
        }
        for c in range(N_CORES)
    ]
    res = run_bass_kernel_spmd(nc, in_maps, list(range(N_CORES)))
    return np.concatenate([res.results[c]["out"] for c in range(N_CORES)], axis=0)


# revision 7
# speedup vs baseline: 1.1944x; 1.1944x over previous
"""Trainium2 Bass kernel for the ConvE-style MoE-routing block.

Computes, for each batch row b:
    X = [e1|e2] @ rel_emb.T            # [B, NR] gating logits
    S, idx = top_k(sigmoid(X), 16)
    R1 = relu(rel_emb @ W_fcs.T + b)   # [NR, D]
    out = sum_k S_k * R1[idx_k] / sum_k S_k

Reformulated gather-free: zap the top-16 logits per row with two
(max8 + match_replace) rounds, then M = sigmoid(X) - sigmoid(X_zapped)
is exactly the top-16 sigmoid weights (0 elsewhere), so
    out = (M @ R1) / rowsum(M)
runs on the tensor engine as a dense matmul.

Data-parallel over batch across 8 cores; rel_emb/W_fcs replicated.
"""
import numpy as np

import concourse.bacc as bacc
import concourse.mybir as mybir
from concourse.bass_utils import run_bass_kernel_spmd
from concourse.masks import make_identity
from concourse.tile import TileContext

P = 128
D = 512
TWO_D = 1024
NR = 2048
B = 8192
N_CORES = 8
BC = B // N_CORES      # 1024 batch rows per core
RT = BC // P           # 8 row tiles per core
KC = TWO_D // P        # 8 feature (contraction) chunks
NRC = NR // P          # 16 rel chunks
NBANK = NR // 512      # 4 PSUM banks for one X row-tile
NEG = -60.0            # sigmoid(x - anything <= NEG+max|x|) == 0 to fp32

F32 = mybir.dt.float32
F32R = mybir.dt.float32r
AF = mybir.ActivationFunctionType

_CACHED = None


def _build():
    nc = bacc.Bacc("TRN2", target_bir_lowering=False, debug=True)
    e1 = nc.declare_dram_parameter("e1", [BC, D], F32, isOutput=False)
    e2 = nc.declare_dram_parameter("e2", [BC, D], F32, isOutput=False)
    rel = nc.declare_dram_parameter("rel_emb", [NR, TWO_D], F32, isOutput=False)
    wf = nc.declare_dram_parameter("W_fcs", [D, TWO_D], F32, isOutput=False)
    bf = nc.declare_dram_parameter("b_fcs", [1, D], F32R, isOutput=False)
    out = nc.declare_dram_parameter("out", [BC, D], F32, isOutput=True)

    # Alternate PSUM->SBUF transpose evictions between ACT and DVE to
    # balance engine load.
    evict_ctr = [0]

    def evict(dst, src):
        if evict_ctr[0] % 2 == 0:
            nc.scalar.activation(dst, src, AF.Copy)
        else:
            nc.vector.tensor_copy(dst, src)
        evict_ctr[0] += 1

    with TileContext(nc) as tc:
        with (
            tc.tile_pool(name="consts", bufs=1) as consts,
            tc.tile_pool(name="persist", bufs=1) as persist,
            tc.tile_pool(name="pst", bufs=2, space="PSUM") as pst,
            tc.tile_pool(name="psx", bufs=1, space="PSUM") as psx,
            tc.tile_pool(name="pso", bufs=2, space="PSUM") as pso,
        ):
            ident = consts.tile([P, P], F32)
            make_identity(nc, ident)
            ones1_f32 = consts.tile([1, P], F32)
            nc.vector.memset(ones1_f32, 1.0)
            ones1 = consts.tile([1, P], F32R)
            nc.vector.tensor_copy(ones1, ones1_f32)
            b_sb = consts.tile([1, D], F32R)
            nc.sync.dma_start(out=b_sb, in_=bf[:])

            # R^T: chunk k (features k*128..) lives at cols [k*NR, (k+1)*NR)
            rt_sb = persist.tile([P, KC * NR], F32)
            # W^T: chunk k at cols [k*D, (k+1)*D)
            wt_sb = persist.tile([P, KC * D], F32R)
            # R1: rel-chunk c at cols [c*D, (c+1)*D)
            r1_sb = persist.tile([P, NRC * D], F32R)

            with tc.tile_pool(name="pre", bufs=2) as pre:
                for a in range(D // P):
                    w_tmp = pre.tile([P, TWO_D], F32, tag="w_tmp")
                    nc.sync.dma_start(out=w_tmp, in_=wf[a * P:(a + 1) * P, :])
                    for k in range(KC):
                        pt = pst.tile([P, P], F32)
                        nc.tensor.transpose(pt, w_tmp[:, k * P:(k + 1) * P], ident)
                        evict(wt_sb[:, k * D + a * P: k * D + (a + 1) * P], pt)
                # R^T blocks, evicted twice: fp32 (gating) + f32r (R1 lhsT,
                # since f32r matmul operands must be produced as f32r). The
                # R1 matmuls for rel-chunk c run as soon as its blocks land.
                for c in range(NRC):
                    r_tmp = pre.tile([P, TWO_D], F32, tag="r_tmp")
                    nc.sync.dma_start(out=r_tmp, in_=rel[c * P:(c + 1) * P, :])
                    rstage = pre.tile([P, KC * P], F32R, tag="rstage")
                    for k in range(KC):
                        pt = pst.tile([P, P], F32)
                        nc.tensor.transpose(pt, r_tmp[:, k * P:(k + 1) * P], ident)
                        evict(rt_sb[:, k * NR + c * P: k * NR + (c + 1) * P], pt)
                        evict(rstage[:, k * P:(k + 1) * P], pt)
                    # R1 = relu(R @ W^T + b), float32r (value-grade).
                    pr = pso.tile([P, D], F32, tag="pso")
                    for k in range(KC):
                        nc.tensor.matmul(
                            pr,
                            lhsT=rstage[:, k * P:(k + 1) * P],
                            rhs=wt_sb[:, k * D:(k + 1) * D],
                            start=(k == 0),
                            stop=False,
                        )
                    nc.tensor.matmul(
                        pr, lhsT=ones1, rhs=b_sb, start=False, stop=True,
                    )
                    nc.scalar.activation(r1_sb[:, c * D:(c + 1) * D], pr, AF.Relu)

            with tc.tile_pool(name="work", bufs=2) as work:
                # Software pipeline: tile m's combine work (M^T transposes +
                # combine matmul) is emitted AFTER tile m+1's gating, so the
                # PE never waits in FIFO order on the serial DVE top-k chain
                # (it is busy with the next tile's gating while the chain
                # runs on DVE/ACT).
                pending = None

                def combine_phase(mm, xs, rec):
                    # M^T (rel on partitions): chunk c at cols [c*P, (c+1)*P)
                    mt = work.tile([P, NRC * P], F32R, tag="mt")
                    for c in range(NRC):
                        pt = pst.tile([P, P], F32)
                        nc.tensor.transpose(pt, xs[:, c * P:(c + 1) * P], ident)
                        evict(mt[:, c * P:(c + 1) * P], pt)
                    # Combine: out2 = M @ R1 (float32r), scaled by 1/denom.
                    op = pso.tile([P, D], F32, tag="pso")
                    for c in range(NRC):
                        nc.tensor.matmul(
                            op,
                            lhsT=mt[:, c * P:(c + 1) * P],
                            rhs=r1_sb[:, c * D:(c + 1) * D],
                            start=(c == 0),
                            stop=(c == NRC - 1),
                        )
                    ot = work.tile([P, D], F32, tag="ot")
                    nc.scalar.activation(ot, op, AF.Copy, scale=rec)
                    nc.sync.dma_start(out=out[mm * P:(mm + 1) * P, :], in_=ot)

                for m in range(RT):
                    st = work.tile([P, TWO_D], F32, tag="st")
                    nc.sync.dma_start(out=st[:, :D], in_=e1[m * P:(m + 1) * P, :])
                    nc.sync.dma_start(out=st[:, D:], in_=e2[m * P:(m + 1) * P, :])
                    # stacked^T: feature-chunk k at cols [k*P, (k+1)*P)
                    stt = work.tile([P, TWO_D], F32, tag="stt")
                    for k in range(KC):
                        pt = pst.tile([P, P], F32)
                        nc.tensor.transpose(pt, st[:, k * P:(k + 1) * P], ident)
                        evict(stt[:, k * P:(k + 1) * P], pt)

                    # Gating X = stacked @ R^T, fp32 (selection-grade).
                    xp = psx.tile([P, NR], F32, tag="xp")
                    for k in range(KC):
                        for nb in range(NBANK):
                            nc.tensor.matmul(
                                xp[:, nb * 512:(nb + 1) * 512],
                                lhsT=stt[:, k * P:(k + 1) * P],
                                rhs=rt_sb[:, k * NR + nb * 512: k * NR + (nb + 1) * 512],
                                start=(k == 0),
                                stop=(k == KC - 1),
                            )
                    xs = work.tile([P, NR], F32, tag="xs")
                    for nb in range(NBANK):
                        nc.scalar.activation(
                            xs[:, nb * 512:(nb + 1) * 512],
                            xp[:, nb * 512:(nb + 1) * 512], AF.Copy,
                        )

                    # Zap top-16 values.
                    m1 = work.tile([P, 8], F32, tag="m1")
                    nc.vector.max(out=m1, in_=xs)
                    xz = work.tile([P, NR], F32, tag="xz")
                    nc.vector.match_replace(
                        out=xz, in_to_replace=m1, in_values=xs, imm_value=NEG)
                    m2 = work.tile([P, 8], F32, tag="m2")
                    nc.vector.max(out=m2, in_=xz)
                    nc.vector.match_replace(
                        out=xz, in_to_replace=m2, in_values=xz, imm_value=NEG)

                    # M = sigmoid(X) - sigmoid(X_zapped); denom via accum.
                    acc_all = work.tile([P, 1], F32, tag="acc_all")
                    nc.scalar.activation(xs, xs, AF.Sigmoid, accum_out=acc_all)
                    acc_exc = work.tile([P, 1], F32, tag="acc_exc")
                    nc.scalar.activation(xz, xz, AF.Sigmoid, accum_out=acc_exc)
                    nc.vector.tensor_sub(xs, xs, xz)
                    den = work.tile([P, 1], F32, tag="den")
                    nc.vector.tensor_sub(den, acc_all, acc_exc)
                    rec = work.tile([P, 1], F32, tag="rec")
                    nc.vector.reciprocal(rec, den)

                    if pending is not None:
                        combine_phase(*pending)
                    pending = (m, xs, rec)
                combine_phase(*pending)

    nc.finalize()
    return nc


def _get_nc():
    global _CACHED
    if _CACHED is None:
        _CACHED = _build()
    return _CACHED


def kernel(e1, e2, rel_emb, W_fcs, b_fcs, **_ignored):
    e1 = np.ascontiguousarray(np.asarray(e1, dtype=np.float32))
    e2 = np.ascontiguousarray(np.asarray(e2, dtype=np.float32))
    rel_emb = np.ascontiguousarray(np.asarray(rel_emb, dtype=np.float32))
    W_fcs = np.ascontiguousarray(np.asarray(W_fcs, dtype=np.float32))
    b_fcs = np.ascontiguousarray(
        np.asarray(b_fcs, dtype=np.float32).reshape(1, D))

    nc = _get_nc()
    in_maps = [
        {
            "e1": e1[c * BC:(c + 1) * BC],
            "e2": e2[c * BC:(c + 1) * BC],
            "rel_emb": rel_emb,
            "W_fcs": W_fcs,
            "b_fcs": b_fcs,
        }
        for c in range(N_CORES)
    ]
    res = run_bass_kernel_spmd(nc, in_maps, list(range(N_CORES)))
    return np.concatenate([res.results[c]["out"] for c in range(N_CORES)], axis=0)
